# revision 1
# baseline (speedup 1.0000x reference)
"""Longformer-style BERT (banded + global attention), 2 layers, on 8 TRN2
NeuronCores via Bass/Tile. Sequence-parallel: each core owns 512 tokens.

Per-core scheme (T=512 local tokens, E=1024 extended key window):
  - residual h: bf16 in DRAM (h_loc); LN math in fp32 on SBUF
  - matmul activations: feature-major bf16 (hT, qT, kT, kgfT, qgT, kgT, oT, gT)
  - v / vgf / vg: token-major bf16 with a ones-column per head ([.., 65*12])
    so PV matmuls produce softmax denominators in psum column 64.
  - band attention: block-banded over 128-token tiles; query tile qt attends
    extended key tiles qt..qt+4, with two static triangle masks on the edge
    tiles and host-computed per-key bias (-30) for invalid keys (out of
    range / global positions / padding).
  - cross-core: AllGather of h (bf16) per layer; halo + global-token rows are
    re-read from the gathered buffer by indirect row gather with host-built
    per-core index vectors. Global-query attention is computed distributed
    (each core scores its own 512 keys) and combined with one small AllReduce.
"""
import os
import sys

sys.path.insert(0, '/opt/trn_rl_repo')
sys.path.insert(0, os.path.dirname(os.path.abspath(__file__)))

import numpy as np
import ml_dtypes

import concourse.bass as bass
import concourse.tile as tile
from concourse import mybir
from concourse.bass_utils import run_bass_kernel_spmd

# ---- walrus sync-wait-limit workaround (inlined) ----
"""Workarounds for the pinned walrus build's per-instruction sync-wait limit.

This walrus errors with 'Too many sync wait commands' when an instruction
carries more than one sem wait. Two patches:

1. TileContext._lower_ordered_insts — before lowering, split any instruction
   with >MAXW on_wait entries: excess waits move to InstNoOp instructions
   inserted just before it on the same engine (engines are in-order, so
   waiting earlier on the same engine is always sound).

2. TileContext._drain_and_barrier — the end-of-kernel drain gets its waits
   spread over SP nops the same way.
"""
import concourse.tile as _tile
from concourse import mybir as _mybir
from concourse.vector_clock import ScopedClock as _ScopedClock

_MAXW = 1


def _split_waits_in_ordered(tc, ordered):
    nc = tc.nc
    for bb_name, insts in ordered.items():
        new_list = []
        for inst in insts:
            si = inst.sync_info
            waits = list(si.on_wait) if si is not None and si.on_wait else []
            if len(waits) > _MAXW and inst.engine != _mybir.EngineType.Unassigned:
                keep = waits[:_MAXW]
                extra = waits[_MAXW:]
                for j in range(0, len(extra), _MAXW):
                    nop = _mybir.InstNoOp(
                        name=nc.get_next_instruction_name(),
                        engine=inst.engine,
                        ins=[],
                        outs=[],
                        sync_info=_mybir.SyncInfo(
                            on_wait=extra[j:j + _MAXW], on_update=[]
                        ),
                        bass_nofuse=True,
                    )
                    nc.register_instruction(nop, overwrite=True)
                    new_list.append(nop)
                inst.sync_info = _mybir.SyncInfo(
                    on_wait=keep,
                    on_update=list(si.on_update) if si.on_update else [],
                )
            new_list.append(inst)
        ordered[bb_name] = new_list


_orig_lower = _tile.TileContext._lower_ordered_insts


def _patched_lower(self, ordered):
    _split_waits_in_ordered(self, ordered)
    return _orig_lower(self, ordered)


_tile.TileContext._lower_ordered_insts = _patched_lower


def _patched_drain_and_barrier(self, tick_clock, wait_clock):
    nc = self.nc
    drain_inst = nc.sync.drain()
    wait_clock.add_sem_waits(
        drain_inst.ins, _ScopedClock({None: tick_clock.global_clock})
    )
    si = drain_inst.ins.sync_info
    waits = list(si.on_wait) if si is not None and si.on_wait else []
    if len(waits) > _MAXW:
        drain_inst.ins.sync_info = _mybir.SyncInfo(
            on_wait=waits[:_MAXW],
            on_update=list(si.on_update) if si.on_update else [],
        )
        for i in range(_MAXW, len(waits), _MAXW):
            nop = nc.sync.nop(nofuse=True)
            nsi = nop.ins.sync_info
            nop.ins.sync_info = _mybir.SyncInfo(
                on_wait=waits[i:i + _MAXW],
                on_update=(list(nsi.on_update)
                           if (nsi is not None and nsi.on_update) else []),
            )
    nc.all_engine_barrier()
    assert self.sems is not None
    popped = nc._tile_sem_poison_stack.pop()
    assert popped is self._sem_poison
    nc.clear_and_free_semaphores(list(self.sems.allocated().values()))
    nc.all_engine_barrier()


_tile.TileContext._drain_and_barrier = _patched_drain_and_barrier


F32 = mybir.dt.float32
BF16 = mybir.dt.bfloat16
I32 = mybir.dt.int32
AF = mybir.ActivationFunctionType
AX = mybir.AxisListType
OP = mybir.AluOpType

NC_ = 8           # cores
S = 4096
D = 768
H = 12
FF = 3072
L = 2
T = S // NC_      # 512 tokens per core
QT = T // 128     # 4 query tiles per core
DT = D // 128     # 6 feature tiles
FT = FF // 128    # 24 ff tiles
ET = QT + 4       # 8 extended key tiles (halo 2 each side)
E = ET * 128      # 1024
SCALE = 1.0 / 8.0
NEG = -30.0
EPS = 1e-5

bfd = ml_dtypes.bfloat16


# ----------------------------------------------------------------------------
# device program
# ----------------------------------------------------------------------------

def build_program():
    nc = bass.Bass()

    def inp(name, shape, dtype=F32):
        return nc.declare_dram_parameter(name, list(shape), dtype,
                                         isOutput=False)

    t = {}
    t["e_word"] = inp("e_word", [T, D])
    t["e_pos"] = inp("e_pos", [T, D])
    t["e_type"] = inp("e_type", [T, D])
    for w in ("Wq", "Wk", "Wv", "Wqg", "Wkg", "Wvg", "Wo"):
        t[w] = inp(w, [L, D, D], BF16)
    t["Wf1"] = inp("Wf1", [L, D, FF], BF16)
    t["Wf2"] = inp("Wf2", [L, FF, D], BF16)
    for b in ("bq_p", "bk_p", "bkg_p", "bqg_p"):
        t[b] = inp(b, [L, 128, DT])
    t["bf1_p"] = inp("bf1_p", [L, 128, FT])
    for b in ("bv_b", "bvg_b", "bo_b", "bf2_b"):
        t[b] = inp(b, [L, 128, D], BF16)
    t["lnes_b"] = inp("lnes_b", [128, D], BF16)
    t["lneb_b"] = inp("lneb_b", [128, D], BF16)
    for b in ("ln1s_b", "ln1b_b", "ln2s_b", "ln2b_b"):
        t[b] = inp(b, [L, 128, D], BF16)
    t["halo_idx"] = inp("halo_idx", [128, 4], I32)
    t["hg_idx"] = inp("hg_idx", [64, 1], I32)
    t["kval_bias"] = inp("kval_bias", [128, ET])
    t["kval01"] = inp("kval01", [128, ET])
    t["gkey_bias"] = inp("gkey_bias", [64, 1])
    t["fkey_bias"] = inp("fkey_bias", [128, QT])
    t["glb1m"] = inp("glb1m", [128, QT])
    t["sel"] = inp("sel", [QT, 64, 128], BF16)
    t["tri_lo"] = inp("tri_lo", [128, 128], BF16)
    t["tri_hi"] = inp("tri_hi", [128, 128], BF16)
    t["ident"] = inp("ident", [128, 128], BF16)
    t["out"] = nc.declare_dram_parameter("out", [T, D], F32, isOutput=True)

    with tile.TileContext(nc) as tc:
        with (
            tc.tile_pool(name="cn", bufs=1) as cn,
            tc.tile_pool(name="wp", bufs=1) as wp,
            tc.tile_pool(name="act", bufs=1) as act,
            tc.tile_pool(name="scr", bufs=1) as scr,
            tc.tile_pool(name="pTp", bufs=1) as pTp,
            tc.tile_pool(name="psp", bufs=1, space="PSUM") as psp,
            tc.tile_pool(name="dram", bufs=1, space="DRAM") as dram,
        ):
            _body(nc, t, cn, wp, act, scr, pTp, psp, dram)
    return nc


def _body(nc, t, cn, wp, act, scr, pTp, psp, dram):
    def load_const(name, shape, dtype=F32):
        tl = cn.tile(list(shape), dtype, tag=name, name=name + "_sb")
        nc.sync.dma_start(tl[:], t[name][:])
        return tl

    tri_lo = load_const("tri_lo", [128, 128], BF16)
    tri_hi = load_const("tri_hi", [128, 128], BF16)
    ident = load_const("ident", [128, 128], BF16)
    halo_idx = load_const("halo_idx", [128, 4], I32)
    hg_idx = load_const("hg_idx", [64, 1], I32)
    kval_bias = load_const("kval_bias", [128, ET])
    kval01 = load_const("kval01", [128, ET])
    gkey_bias = load_const("gkey_bias", [64, 1])
    fkey_bias = load_const("fkey_bias", [128, QT])
    glb1m = load_const("glb1m", [128, QT])
    lnes_b = load_const("lnes_b", [128, D], BF16)
    lneb_b = load_const("lneb_b", [128, D], BF16)
    sel_sb = cn.tile([64, QT, 128], BF16, tag="sel", name="sel_sb")
    nc.sync.dma_start(sel_sb[:], t["sel"].rearrange("q g t -> g q t")[:])
    eps_c = cn.tile([128, 1], F32, tag="eps_c", name="eps_c")
    nc.vector.memset(eps_c[:], EPS)

    def big32(name="b32"):
        return scr.tile([128, D], F32, tag="sD32", bufs=3, name=name)

    def small32(name="s32"):
        return scr.tile([128, 1], F32, tag="s1", bufs=6, name=name)

    def bigbf(name="bbf"):
        return scr.tile([128, D], BF16, tag="sDbf", bufs=2, name=name)

    # ---- layernorm: x fp32 [128, D] -> out_ap; var = E[x^2] - mean^2
    def layer_norm(x_tile, s_b, b_b, out_ap):
        red = small32("ln_red")
        nc.vector.tensor_reduce(red[:], x_tile[:], axis=AX.X, op=OP.add)
        mean = small32("ln_mean")
        nc.scalar.mul(mean[:], red[:], 1.0 / D)
        sq = scr.tile([128, D], F32, tag="ln_sq", bufs=1, name="ln_sq")
        ssq = small32("ln_ssq")
        nc.scalar.activation(sq[:], x_tile[:], AF.Square,
                             accum_out=ssq[:, 0:1])
        # bias = eps - mean^2
        vb = small32("ln_vb")
        nc.vector.tensor_scalar(vb[:], mean[:], mean[:, 0:1], -1.0,
                                op0=OP.mult, op1=OP.mult)
        nc.vector.tensor_scalar_add(vb[:], vb[:], eps_c[:, 0:1])
        std = small32("ln_std")
        nc.scalar.activation(std[:], ssq[:], AF.Sqrt, bias=vb[:, 0:1],
                             scale=1.0 / D)
        rstd = small32("ln_rstd")
        nc.vector.reciprocal(rstd[:], std[:])
        zn = big32("ln_zn")
        nc.vector.tensor_scalar(zn[:], x_tile[:], mean[:, 0:1],
                                rstd[:, 0:1], op0=OP.subtract, op1=OP.mult)
        tmp = big32("ln_tmp")
        nc.vector.tensor_mul(tmp[:], zn[:], s_b[:])
        nc.vector.tensor_add(out_ap, tmp[:], b_b[:])

    h_loc = dram.tile([T, D], BF16, tag="h_loc", name="h_loc")
    h1 = [act.tile([128, D], F32, tag=f"h1_{i}", name=f"h1_{i}")
          for i in range(QT)]
    h_bf = [act.tile([128, D], BF16, tag=f"hbf_{i}", name=f"hbf_{i}")
            for i in range(QT)]

    # ---- embedding -> h_loc (bf16)
    for i in range(QT):
        sl = slice(i * 128, (i + 1) * 128)
        ew = big32("emb_w")
        ep = big32("emb_p")
        et_ = scr.tile([128, D], F32, tag="emb_t", bufs=2, name="emb_t")
        nc.sync.dma_start(ew[:], t["e_word"][sl, :])
        nc.scalar.dma_start(ep[:], t["e_pos"][sl, :])
        nc.sync.dma_start(et_[:], t["e_type"][sl, :])
        s1 = big32("emb_s1")
        nc.vector.tensor_add(s1[:], ew[:], ep[:])
        s2 = big32("emb_s2")
        nc.vector.tensor_add(s2[:], s1[:], et_[:])
        layer_norm(s2, lnes_b, lneb_b, h_bf[i][:])
        nc.sync.dma_start(h_loc[sl, :], h_bf[i][:])

    # ---------------- layers ----------------
    for l in range(L):
        # ---- C1 first: AllGather h (bf16)
        h_full = dram.tile([S, D], BF16, tag="h_full", name=f"h_full{l}",
                           addr_space="Shared")
        nc.gpsimd.collective_compute(
            "AllGather", OP.bypass,
            ins=[h_loc[:]], outs=[h_full[:]],
            replica_groups=[list(range(NC_))],
        )

        # ---- hT_own via PE transposes (transposing DMAs would serialize
        # against the collective)
        hT_own = act.tile([128, DT, T], BF16, tag="hT_own", name=f"hTo{l}")
        for i in range(QT):
            for d in range(DT):
                tps = psp.tile([128, 128], BF16, tag="p", bufs=8, name="trh")
                nc.tensor.transpose(tps[:], h_bf[i][:, d * 128:(d + 1) * 128],
                                    ident[:])
                nc.vector.tensor_copy(hT_own[:, d, i * 128:(i + 1) * 128],
                                      tps[:])

        def wslab(src_ap, name, eng=None):
            tl = wp.tile([128, DT, D], BF16, tag="w", bufs=7, name=name)
            (eng or nc.sync).dma_start(
                tl[:], src_ap.rearrange("(k p) o -> p k o", p=128)[:])
            return tl

        w_q = wslab(t["Wq"][l], f"wq{l}")
        w_k = wslab(t["Wk"][l], f"wk{l}")
        w_v = wslab(t["Wv"][l], f"wv{l}")
        w_qg = wslab(t["Wqg"][l], f"wqg{l}")
        w_kg = wslab(t["Wkg"][l], f"wkg{l}")
        w_vg = wslab(t["Wvg"][l], f"wvg{l}")
        w_o = wslab(t["Wo"][l], f"wo{l}")

        def bload(name, n=DT, dtype=F32):
            tl = wp.tile([128, n], dtype, tag=f"b_{name}", name=f"{name}{l}")
            nc.scalar.dma_start(tl[:], t[name][l][:])
            return tl

        b_q = bload("bq_p")
        b_k = bload("bk_p")
        b_kg = bload("bkg_p")
        b_qg = bload("bqg_p")
        b_f1 = bload("bf1_p", FT)
        b_v = bload("bv_b", D, BF16)
        b_vg = bload("bvg_b", D, BF16)
        b_o = bload("bo_b", D, BF16)
        b_f2 = bload("bf2_b", D, BF16)
        ln1s = bload("ln1s_b", D, BF16)
        ln1b = bload("ln1b_b", D, BF16)
        ln2s = bload("ln2s_b", D, BF16)
        ln2b = bload("ln2b_b", D, BF16)

        # ---- projections; rhs given as chunks (tile, out_col0, width)
        def proj_chunk(w_sb, b_sb, o, rhs, col0, w):
            for cc in range(0, w, 512):
                cw = min(cc + 512, w) - cc
                for ot in range(DT):
                    ps = psp.tile([128, 512], F32, tag="p", bufs=8,
                                  name="pw")
                    for k in range(DT):
                        nc.tensor.matmul(
                            ps[:, 0:cw],
                            w_sb[:, k, ot * 128:(ot + 1) * 128],
                            rhs[:, k, cc:cc + cw],
                            start=(k == 0), stop=(k == DT - 1))
                    nc.vector.tensor_scalar_add(
                        o[:, ot, col0 + cc:col0 + cc + cw], ps[:, 0:cw],
                        b_sb[:, ot:ot + 1])

        def proj_tm_tiles(w_sb, b_bc, o, tts, rhs_of):
            for tt in tts:
                rhs, tcol = rhs_of(tt)
                for c0 in (0, 512):
                    c1 = min(c0 + 512, D)
                    ps = psp.tile([128, 512], F32, tag="p", bufs=8, name="pt")
                    for k in range(DT):
                        nc.tensor.matmul(
                            ps[:, 0:c1 - c0],
                            rhs[:, k, tcol:tcol + 128],
                            w_sb[:, k, c0:c1],
                            start=(k == 0), stop=(k == DT - 1))
                    biased = big32("ptb")
                    nc.vector.tensor_add(biased[:, 0:c1 - c0],
                                         ps[:, 0:c1 - c0], b_bc[:, c0:c1])
                    nh = (c1 - c0) // 64
                    h0 = c0 // 64
                    dst = o[:, tt].rearrange("p (hh c) -> p hh c", c=65)
                    nc.vector.tensor_copy(
                        dst[:, h0:h0 + nh, 0:64],
                        biased[:, 0:c1 - c0]
                        .rearrange("p (hh c) -> p hh c", c=64)[:])
                nc.vector.memset(
                    o[:, tt].rearrange("p (hh c) -> p hh c", c=65)
                    [:, :, 64:65], 1.0)

        # ---- local-only projections first (overlap the AllGather)
        qT = act.tile([128, DT, T], BF16, tag="qT", name=f"qT{l}")
        proj_chunk(w_q, b_q, qT, hT_own, 0, T)
        kgfT = act.tile([128, DT, T], BF16, tag="kgfT", name=f"kgfT{l}")
        proj_chunk(w_kg, b_kg, kgfT, hT_own, 0, T)
        kT = act.tile([128, DT, E], BF16, tag="kT", name=f"kT{l}")
        proj_chunk(w_k, b_k, kT, hT_own, 256, T)
        v_sb = act.tile([128, ET, H * 65], BF16, tag="v_sb", name=f"v{l}")
        vgf_sb = act.tile([128, QT, H * 65], BF16, tag="vgf_sb",
                          name=f"vgf{l}")

        def v_rhs(tt):
            if tt < 2:
                return hT_hl, tt * 128
            if tt < 6:
                return hT_own, (tt - 2) * 128
            return hT_hr, (tt - 6) * 128

        hT_hl = act.tile([128, DT, 256], BF16, tag="hT_hl", name=f"hl{l}")
        hT_hr = act.tile([128, DT, 256], BF16, tag="hT_hr", name=f"hr{l}")
        proj_tm_tiles(w_v, b_v, v_sb, [2, 3, 4, 5], v_rhs)
        proj_tm_tiles(w_vg, b_vg, vgf_sb, list(range(QT)),
                      lambda tt: (hT_own, tt * 128))

        # ---- halo tiles from h_full (gather + PE transpose)
        for g in range(4):  # 0,1 left; 2,3 right
            htmp = bigbf(f"halo{g}")
            nc.gpsimd.indirect_dma_start(
                out=htmp[:], out_offset=None, in_=h_full[:],
                in_offset=bass.IndirectOffsetOnAxis(
                    ap=halo_idx[:, g:g + 1], axis=0),
            )
            dst, off = (hT_hl, g * 128) if g < 2 else (hT_hr, (g - 2) * 128)
            for d in range(DT):
                tps = psp.tile([128, 128], BF16, tag="p", bufs=8, name="trp")
                nc.tensor.transpose(tps[:], htmp[:, d * 128:(d + 1) * 128],
                                    ident[:])
                nc.vector.tensor_copy(dst[:, d, off:off + 128], tps[:])
        # ---- hgT [128, DT, 64]
        hg_tm = bigbf("hg_tm")
        nc.gpsimd.indirect_dma_start(
            out=hg_tm[0:64, :], out_offset=None, in_=h_full[:],
            in_offset=bass.IndirectOffsetOnAxis(ap=hg_idx[:, 0:1], axis=0),
        )
        hgT = act.tile([128, DT, 64], BF16, tag="hgT", name=f"hgT{l}")
        for d in range(DT):
            tps = psp.tile([128, 128], BF16, tag="p", bufs=8, name="trg")
            nc.tensor.transpose(tps[:, 0:64],
                                hg_tm[0:64, d * 128:(d + 1) * 128],
                                ident[0:64, 0:64])
            nc.vector.tensor_copy(hgT[:, d, :], tps[:, 0:64])

        # ---- halo-dependent projection parts
        proj_chunk(w_k, b_k, kT, hT_hl, 0, 256)
        proj_chunk(w_k, b_k, kT, hT_hr, 768, 256)
        proj_tm_tiles(w_v, b_v, v_sb, [0, 1, 6, 7], v_rhs)
        # fold key-validity masking into v (zero rows + ones entries of
        # invalid keys) so band exps need no per-key bias
        for e in range(ET):
            nc.vector.tensor_scalar_mul(v_sb[:, e, :], v_sb[:, e, :],
                                        kval01[:, e:e + 1])

        def proj_fm_g(w_sb, b_sb, tag):
            o = act.tile([128, DT, 64], BF16, tag=tag, name=tag + str(l))
            for ot in range(DT):
                ps = psp.tile([128, 512], F32, tag="p", bufs=8, name="pg_")
                for k in range(DT):
                    nc.tensor.matmul(
                        ps[:, 0:64], w_sb[:, k, ot * 128:(ot + 1) * 128],
                        hgT[:, k, :],
                        start=(k == 0), stop=(k == DT - 1))
                nc.vector.tensor_scalar_add(o[:, ot, :], ps[:, 0:64],
                                            b_sb[:, ot:ot + 1])
            return o

        qgT = proj_fm_g(w_qg, b_qg, "qgT")
        kgT = proj_fm_g(w_k, b_k, "kgT")

        vg_sb = act.tile([64, H * 65], BF16, tag="vg_sb", name=f"vg{l}")
        for c0 in (0, 512):
            c1 = min(c0 + 512, D)
            ps = psp.tile([128, 512], F32, tag="p", bufs=8, name="pvg")
            for k in range(DT):
                nc.tensor.matmul(ps[0:64, 0:c1 - c0], hgT[:, k, :],
                                 w_v[:, k, c0:c1],
                                 start=(k == 0), stop=(k == DT - 1))
            biased = big32("vgb")
            nc.vector.tensor_add(biased[0:64, 0:c1 - c0], ps[0:64, 0:c1 - c0],
                                 b_v[0:64, c0:c1])
            nh = (c1 - c0) // 64
            h0 = c0 // 64
            dst = vg_sb.rearrange("p (hh c) -> p hh c", c=65)
            nc.vector.tensor_copy(
                dst[:, h0:h0 + nh, 0:64],
                biased[0:64, 0:c1 - c0]
                .rearrange("p (hh c) -> p hh c", c=64)[:])
        nc.vector.memset(
            vg_sb.rearrange("p (hh c) -> p hh c", c=65)[:, :, 64:65], 1.0)

        # ---- global-query attention partials + AllReduce (issued early so
        # the collective overlaps the band attention below)
        stag = scr.tile([64, H, 65], F32, tag="gq_stage", name=f"stag{l}")
        for hh in range(H):
            hp, hr = hh // 2, (hh % 2) * 64
            prow = slice(hr, hr + 64)
            pfs = []
            for kt in range(QT):
                sps = psp.tile([128, 128], F32, tag="p", bufs=8, name="sf")
                nc.tensor.matmul(
                    sps[:, 0:64], kgfT[prow, hp, kt * 128:(kt + 1) * 128],
                    qgT[prow, hp, :], start=True, stop=True)
                pf = pTp.tile([128, 64], BF16, tag="pf", bufs=5, name="pf")
                nc.scalar.activation(pf[:], sps[:, 0:64], AF.Exp,
                                     bias=fkey_bias[:, kt:kt + 1],
                                     scale=SCALE)
                pfs.append(pf)
            gps = psp.tile([128, 65], F32, tag="p", bufs=8, name="gps")
            for kt in range(QT):
                nc.tensor.matmul(gps[0:64, :], pfs[kt][:],
                                 vgf_sb[:, kt, hh * 65:(hh + 1) * 65],
                                 start=(kt == 0), stop=(kt == QT - 1))
            nc.vector.tensor_copy(stag[:, hh, :], gps[0:64, :])
        cc2_in = dram.tile([64, H * 65], F32, tag="cc2_in", name=f"c2i{l}")
        cc2_out = dram.tile([64, H * 65], F32, tag="cc2_out", name=f"c2o{l}",
                            addr_space="Shared")
        nc.sync.dma_start(cc2_in[:], stag.rearrange("p a b -> p (a b)")[:])
        nc.gpsimd.collective_compute(
            "AllReduce", OP.add,
            ins=[cc2_in[:]], outs=[cc2_out[:]],
            replica_groups=[list(range(NC_))],
        )
        gsum = scr.tile([64, H, 65], F32, tag="gq_sum", name=f"gsum{l}")
        nc.sync.dma_start(gsum.rearrange("p a b -> p (a b)")[:], cc2_out[:])

        # ---- band + global-key attention -> o_sb
        o_sb = act.tile([128, QT, D], BF16, tag="o_sb", name=f"osb{l}")
        for qt in range(QT):
            qsl = slice(qt * 128, (qt + 1) * 128)
            for hh in range(H):
                hp, hr = hh // 2, (hh % 2) * 64
                prow = slice(hr, hr + 64)
                sg = psp.tile([128, 128], F32, tag="p", bufs=8, name="sg")
                nc.tensor.matmul(sg[0:64, :], kgT[prow, hp, :],
                                 qT[prow, hp, qsl], start=True, stop=True)
                pg = pTp.tile([64, 128], BF16, tag="pg", bufs=2, name="pg")
                nc.scalar.activation(pg[:], sg[0:64, :], AF.Exp,
                                     bias=gkey_bias[:, 0:1], scale=SCALE)
                sp4 = psp.tile([128, 512], F32, tag="p", bufs=8,
                               name="sp4")
                for a in range(4):
                    e = qt + a
                    nc.tensor.matmul(
                        sp4[:, a * 128:(a + 1) * 128],
                        kT[prow, hp, e * 128:(e + 1) * 128],
                        qT[prow, hp, qsl], start=True, stop=True)
                sp_hi = psp.tile([128, 128], F32, tag="p", bufs=8,
                                 name="sp_hi")
                nc.tensor.matmul(
                    sp_hi[:], kT[prow, hp, (qt + 4) * 128:(qt + 5) * 128],
                    qT[prow, hp, qsl], start=True, stop=True)
                pt4 = pTp.tile([128, 512], BF16, tag="pt4", bufs=8,
                               name="pt4")
                nc.scalar.activation(pt4[:], sp4[:], AF.Exp, scale=SCALE)
                pt_hi = pTp.tile([128, 128], BF16, tag="pth", bufs=4,
                                 name="pth")
                nc.scalar.activation(pt_hi[:], sp_hi[:], AF.Exp, scale=SCALE)
                nc.vector.tensor_mul(pt4[:, 0:128], pt4[:, 0:128], tri_lo[:])
                nc.vector.tensor_mul(pt_hi[:], pt_hi[:], tri_hi[:])
                ops = psp.tile([128, 65], F32, tag="p", bufs=8, name="ops")
                nc.tensor.matmul(ops[:], pg[:],
                                 vg_sb[:, hh * 65:(hh + 1) * 65],
                                 start=True, stop=False)
                for a in range(4):
                    e = qt + a
                    nc.tensor.matmul(
                        ops[:], pt4[:, a * 128:(a + 1) * 128],
                        v_sb[:, e, hh * 65:(hh + 1) * 65],
                        start=False, stop=False)
                nc.tensor.matmul(
                    ops[:], pt_hi[:],
                    v_sb[:, qt + 4, hh * 65:(hh + 1) * 65],
                    start=False, stop=True)
                rec = small32("rec")
                nc.vector.reciprocal(rec[:], ops[:, 64:65])
                fac = small32("fac")
                nc.vector.tensor_mul(fac[:], rec[:], glb1m[:, qt:qt + 1])
                nc.vector.tensor_scalar_mul(
                    o_sb[:, qt, hh * 64:(hh + 1) * 64], ops[:, 0:64],
                    fac[:, 0:1])

        # ---- og from the AllReduce result; scatter into o_sb
        og = act.tile([64, D], BF16, tag="og", name=f"og{l}")
        for hh in range(H):
            rec = small32("grec")
            nc.vector.reciprocal(rec[0:64, :], gsum[:, hh, 64:65])
            nc.vector.tensor_scalar_mul(og[:, hh * 64:(hh + 1) * 64],
                                        gsum[:, hh, 0:64], rec[0:64, 0:1])
        for qt in range(QT):
            for c0 in (0, 512):
                c1 = min(c0 + 512, D)
                sc = psp.tile([128, 512], F32, tag="p", bufs=8, name="sc")
                nc.tensor.matmul(sc[:, 0:c1 - c0], sel_sb[:, qt, :],
                                 og[:, c0:c1], start=True, stop=True)
                nc.vector.tensor_add(o_sb[:, qt, c0:c1], o_sb[:, qt, c0:c1],
                                     sc[:, 0:c1 - c0])

        # ---- oT via PE transposes (shares qT slot)
        oT = act.tile([128, DT, T], BF16, tag="qT", name=f"oT{l}")
        for qt in range(QT):
            for d in range(DT):
                tps = psp.tile([128, 128], BF16, tag="p", bufs=8, name="tro")
                nc.tensor.transpose(
                    tps[:], o_sb[:, qt, d * 128:(d + 1) * 128], ident[:])
                nc.vector.tensor_copy(oT[:, d, qt * 128:(qt + 1) * 128],
                                      tps[:])

        # ---- Wo + residual + LN1 -> h1 (f32) + h1_loc (bf16)
        for qt in range(QT):
            x1 = big32("x1")
            nc.vector.tensor_add(x1[:], h_bf[qt][:], b_o[:])
            for c0 in (0, 512):
                c1 = min(c0 + 512, D)
                ps = psp.tile([128, 512], F32, tag="p", bufs=8, name="pwo")
                for k in range(DT):
                    nc.tensor.matmul(
                        ps[:, 0:c1 - c0], oT[:, k, qt * 128:(qt + 1) * 128],
                        w_o[:, k, c0:c1],
                        start=(k == 0), stop=(k == DT - 1))
                nc.vector.tensor_add(x1[:, c0:c1], x1[:, c0:c1],
                                     ps[:, 0:c1 - c0])
            layer_norm(x1, ln1s, ln1b, h1[qt][:])

        # ---- h1T via PE transposes (shares kgfT slot)
        h1T = act.tile([128, DT, T], BF16, tag="kgfT", name=f"h1T{l}")
        for qt in range(QT):
            h1b = bigbf(f"h1b{qt}")
            nc.vector.tensor_copy(h1b[:], h1[qt][:])
            for d in range(DT):
                tps = psp.tile([128, 128], BF16, tag="p", bufs=8, name="trh1")
                nc.tensor.transpose(tps[:], h1b[:, d * 128:(d + 1) * 128],
                                    ident[:])
                nc.vector.tensor_copy(h1T[:, d, qt * 128:(qt + 1) * 128],
                                      tps[:])

        # ---- FFN: x2 accumulates in-place on h1 (f32)
        for qt in range(QT):
            nc.vector.tensor_add(h1[qt][:], h1[qt][:], b_f2[:])
        for half in range(2):
            f1a = wslab(t["Wf1"][l][:, half * 1536:half * 1536 + 768],
                        f"f1a{l}{half}", eng=nc.scalar)
            f1b = wslab(t["Wf1"][l][:, half * 1536 + 768:(half + 1) * 1536],
                        f"f1b{l}{half}", eng=nc.scalar)
            f2a = wslab(t["Wf2"][l][half * 1536:half * 1536 + 768, :],
                        f"f2a{l}{half}", eng=nc.scalar)
            f2b = wslab(t["Wf2"][l][half * 1536 + 768:(half + 1) * 1536, :],
                        f"f2b{l}{half}", eng=nc.scalar)
            gT = act.tile([128, FT // 2, T], BF16, tag="v_sb", bufs=1,
                          name=f"gT{l}{half}")
            for ft in range(FT // 2):
                fabs = half * (FT // 2) + ft
                slab = f1a if ft < 6 else f1b
                ps = psp.tile([128, 512], F32, tag="p", bufs=8, name="pf1")
                for k in range(DT):
                    nc.tensor.matmul(
                        ps[:], slab[:, k, (ft % 6) * 128:(ft % 6 + 1) * 128],
                        h1T[:, k, :],
                        start=(k == 0), stop=(k == DT - 1))
                nc.scalar.activation(gT[:, ft, :], ps[:], AF.Gelu_apprx_tanh,
                                     bias=b_f1[:, fabs:fabs + 1])
            for qt in range(QT):
                for c0 in (0, 512):
                    c1 = min(c0 + 512, D)
                    ps = psp.tile([128, 512], F32, tag="p", bufs=8,
                                  name="pf2")
                    for k in range(FT // 2):
                        slab = f2a if k < 6 else f2b
                        nc.tensor.matmul(
                            ps[:, 0:c1 - c0],
                            gT[:, k, qt * 128:(qt + 1) * 128],
                            slab[:, k % 6, c0:c1],
                            start=(k == 0), stop=(k == FT // 2 - 1))
                    nc.vector.tensor_add(h1[qt][:, c0:c1], h1[qt][:, c0:c1],
                                         ps[:, 0:c1 - c0])
        for qt in range(QT):
            if l + 1 < L:
                layer_norm(h1[qt], ln2s, ln2b, h_bf[qt][:])
                nc.sync.dma_start(h_loc[qt * 128:(qt + 1) * 128, :],
                                  h_bf[qt][:])
            else:
                hout = big32("hout")
                layer_norm(h1[qt], ln2s, ln2b, hout[:])
                nc.sync.dma_start(t["out"][qt * 128:(qt + 1) * 128, :],
                                  hout[:])


# ----------------------------------------------------------------------------
# host side
# ----------------------------------------------------------------------------

_prog_cache = {}


def _get_program():
    if "nc" not in _prog_cache:
        _prog_cache["nc"] = build_program()
    return _prog_cache["nc"]


def _prep_maps(inputs):
    gi = {k: np.asarray(v) for k, v in inputs.items()}
    x = gi["x"][0]
    segs = gi["segs"][0]
    mask = gi["mask_src"][0] > 0
    clss = gi["clss"][0]

    is_glb = np.zeros(S, bool)
    is_glb[clss] = True

    def bcast(v, dt=np.float32):
        v = np.asarray(v, np.float32)
        return np.broadcast_to(v[None, :], (128, v.shape[0])).astype(dt)

    def part(v):
        return np.asarray(v, np.float32).reshape(-1, 128).T.copy()

    shared = {
        "Wq": gi["Wq"].astype(bfd), "Wk": gi["Wk"].astype(bfd),
        "Wv": gi["Wv"].astype(bfd), "Wqg": gi["Wqg"].astype(bfd),
        "Wkg": gi["Wkg"].astype(bfd), "Wvg": gi["Wvg"].astype(bfd),
        "Wo": gi["Wo"].astype(bfd),
        "Wf1": gi["Wf1"].astype(bfd), "Wf2": gi["Wf2"].astype(bfd),
        "bq_p": np.stack([part(gi["bq"][l]) for l in range(L)]),
        "bk_p": np.stack([part(gi["bk"][l]) for l in range(L)]),
        "bkg_p": np.stack([part(gi["bkg"][l]) for l in range(L)]),
        "bqg_p": np.stack([part(gi["bqg"][l]) for l in range(L)]),
        "bf1_p": np.stack([part(gi["bf1"][l]) for l in range(L)]),
        "bv_b": np.stack([bcast(gi["bv"][l], bfd) for l in range(L)]),
        "bvg_b": np.stack([bcast(gi["bvg"][l], bfd) for l in range(L)]),
        "bo_b": np.stack([bcast(gi["bo"][l], bfd) for l in range(L)]),
        "bf2_b": np.stack([bcast(gi["bf2"][l], bfd) for l in range(L)]),
        "lnes_b": bcast(gi["ln_e_s"], bfd), "lneb_b": bcast(gi["ln_e_b"], bfd),
        "ln1s_b": np.stack([bcast(gi["ln1_s"][l], bfd) for l in range(L)]),
        "ln1b_b": np.stack([bcast(gi["ln1_b"][l], bfd) for l in range(L)]),
        "ln2s_b": np.stack([bcast(gi["ln2_s"][l], bfd) for l in range(L)]),
        "ln2b_b": np.stack([bcast(gi["ln2_b"][l], bfd) for l in range(L)]),
        "hg_idx": clss.astype(np.int32).reshape(64, 1),
        "gkey_bias": np.where(mask[clss], 0.0, NEG).astype(np.float32)
                       .reshape(64, 1),
        "tri_lo": (np.arange(128)[:, None] >= np.arange(128)[None, :])
                    .astype(bfd),
        "tri_hi": (np.arange(128)[:, None] <= np.arange(128)[None, :])
                    .astype(bfd),
        "ident": np.eye(128, dtype=bfd),
    }

    # scatter representative: one entry per position (duplicates collapse)
    rep = np.zeros(64, bool)
    seen = set()
    for g in range(63, -1, -1):
        if int(clss[g]) not in seen:
            seen.add(int(clss[g]))
            rep[g] = True

    maps = []
    for c in range(NC_):
        s0, s1 = c * T, (c + 1) * T
        toks = np.arange(s0, s1)
        ext = np.arange(s0 - 256, s1 + 256)
        ext_ok = (ext >= 0) & (ext < S)
        extc = np.clip(ext, 0, S - 1)
        kval = np.where(ext_ok & mask[extc] & ~is_glb[extc], 0.0, NEG)
        halo = np.concatenate([extc[:256], extc[-256:]]).astype(np.int32)
        sel = np.zeros((QT, 64, 128), np.float32)
        for g in range(64):
            p = int(clss[g])
            if rep[g] and s0 <= p < s1:
                sel[(p - s0) // 128, g, (p - s0) % 128] = 1.0
        m = {
            "e_word": gi["word_emb"][x[s0:s1]].astype(np.float32),
            "e_pos": gi["pos_emb"][s0:s1].astype(np.float32),
            "e_type": gi["type_emb"][segs[s0:s1]].astype(np.float32),
            "halo_idx": halo.reshape(4, 128).T.copy(),
            "kval_bias": kval.astype(np.float32).reshape(ET, 128).T.copy(),
            "kval01": (kval == 0.0).astype(np.float32)
                        .reshape(ET, 128).T.copy(),
            "fkey_bias": np.where(mask[toks], 0.0, NEG).astype(np.float32)
                           .reshape(QT, 128).T.copy(),
            "glb1m": (~is_glb[toks]).astype(np.float32)
                       .reshape(QT, 128).T.copy(),
            "sel": sel.astype(bfd),
        }
        m.update(shared)
        maps.append(m)
    return maps


def kernel(**inputs):
    nc = _get_program()
    maps = _prep_maps(inputs)
    res = run_bass_kernel_spmd(nc, maps, list(range(NC_)))
    out = np.concatenate([res.results[c]["out"] for c in range(NC_)], axis=0)
    return out[None].astype(np.float32)



# revision 76
# speedup vs baseline: 1.4836x; 1.4836x over previous
"""Longformer-style BERT (banded + global attention), 2 layers, on 8 TRN2
NeuronCores via Bass/Tile. Sequence-parallel: each core owns 512 tokens.

v2 collective scheme (vs v1 full-h AllGather per layer):
  - layer 0: host precomputes h0 = LN(emb) and uploads it transposed and
    halo-extended per core, plus the 64 global rows -> no collectives in
    layer 0 except the small global-query ReduceScatter.
  - layer 1 halo: two pairwise-group AllGathers (256-token contributions,
    staged by indirect row gather so the per-parity row choice is input
    data, keeping the program SPMD-uniform).
  - layer 1 globals: AllGather of each core's owned global-h rows padded to
    M = max-per-core count, owner-ordered (layout baked into the program;
    the program cache is keyed on the per-core counts).
  - global queries: partial softmax per core + ReduceScatter in owner-order
    layout (each core only needs its own tokens' outputs).
Weight DMAs: proj slabs on SP, FFN slabs on Pool emitted after the RS so
they never block collectives; all share one 7-deep SBUF rotation.
"""
import os
import sys

sys.path.insert(0, '/opt/trn_rl_repo')
sys.path.insert(0, os.path.dirname(os.path.abspath(__file__)))

import numpy as np
import ml_dtypes

import concourse.bass as bass
import concourse.tile as tile
from concourse import mybir
from concourse.bass_utils import run_bass_kernel_spmd

# ---- walrus sync-wait-limit workaround (inlined) ----
import concourse.tile as _tile
from concourse import mybir as _mybir
from concourse.vector_clock import ScopedClock as _ScopedClock

_MAXW = 1


def _split_waits_in_ordered(tc, ordered):
    nc = tc.nc
    for bb_name, insts in ordered.items():
        new_list = []
        for inst in insts:
            si = inst.sync_info
            waits = list(si.on_wait) if si is not None and si.on_wait else []
            if len(waits) > _MAXW and inst.engine != _mybir.EngineType.Unassigned:
                keep = waits[:_MAXW]
                extra = waits[_MAXW:]
                for j in range(0, len(extra), _MAXW):
                    nop = _mybir.InstNoOp(
                        name=nc.get_next_instruction_name(),
                        engine=inst.engine,
                        ins=[],
                        outs=[],
                        sync_info=_mybir.SyncInfo(
                            on_wait=extra[j:j + _MAXW], on_update=[]
                        ),
                        bass_nofuse=True,
                    )
                    nc.register_instruction(nop, overwrite=True)
                    new_list.append(nop)
                inst.sync_info = _mybir.SyncInfo(
                    on_wait=keep,
                    on_update=list(si.on_update) if si.on_update else [],
                )
            new_list.append(inst)
        ordered[bb_name] = new_list


_orig_lower = _tile.TileContext._lower_ordered_insts


def _patched_lower(self, ordered):
    _split_waits_in_ordered(self, ordered)
    return _orig_lower(self, ordered)


_tile.TileContext._lower_ordered_insts = _patched_lower


def _patched_drain_and_barrier(self, tick_clock, wait_clock):
    nc = self.nc
    drain_inst = nc.sync.drain()
    wait_clock.add_sem_waits(
        drain_inst.ins, _ScopedClock({None: tick_clock.global_clock})
    )
    si = drain_inst.ins.sync_info
    waits = list(si.on_wait) if si is not None and si.on_wait else []
    if len(waits) > _MAXW:
        drain_inst.ins.sync_info = _mybir.SyncInfo(
            on_wait=waits[:_MAXW],
            on_update=list(si.on_update) if si.on_update else [],
        )
        for i in range(_MAXW, len(waits), _MAXW):
            nop = nc.sync.nop(nofuse=True)
            nsi = nop.ins.sync_info
            nop.ins.sync_info = _mybir.SyncInfo(
                on_wait=waits[i:i + _MAXW],
                on_update=(list(nsi.on_update)
                           if (nsi is not None and nsi.on_update) else []),
            )
    nc.all_engine_barrier()
    assert self.sems is not None
    popped = nc._tile_sem_poison_stack.pop()
    assert popped is self._sem_poison
    nc.clear_and_free_semaphores(list(self.sems.allocated().values()))
    nc.all_engine_barrier()


_tile.TileContext._drain_and_barrier = _patched_drain_and_barrier


F32 = mybir.dt.float32
BF16 = mybir.dt.bfloat16
FP8 = mybir.dt.float8e4
KD = (mybir.dt.bfloat16 if os.environ.get('KHALO_BF16', '0') == '1'
      else mybir.dt.float8e4)
NOPOOL = os.environ.get('NOPOOL', '0') == '1'
NOGPD = os.environ.get('NOGPD', '0') == '1'
LL = 1 if os.environ.get('LONLY', '0') == '1' else 2
NORS = os.environ.get('NORS', '0') == '1'
NOBAND = os.environ.get('NOBAND', '0') == '1'
NOLN = os.environ.get('NOLN', '0') == '1'
NOSG = os.environ.get('NOSG', '0') == '1'
NOHI = os.environ.get('NOHI', '1') == '1'  # merged hi-pair exp faults on hw
I32 = mybir.dt.int32
AF = mybir.ActivationFunctionType
AX = mybir.AxisListType
OP = mybir.AluOpType

NC_ = 8           # cores
S = 4096
D = 768
H = 12
FF = 3072
L = 2
T = S // NC_      # 512 tokens per core
QT = T // 128     # 4 query tiles per core
DT = D // 128     # 6 feature tiles
FT = FF // 128    # 24 ff tiles
ET = QT + 4       # 8 extended key tiles (halo 2 each side)
E = ET * 128      # 1024
SCALE = 1.0 / 8.0
NEG = -30.0
EPS = 1e-5

GRP_A = [[0, 1], [2, 3], [4, 5], [6, 7]]
GRP_B = [[0, 7], [1, 2], [3, 4], [5, 6]]
GRP_ALL = [list(range(NC_))]

bfd = ml_dtypes.bfloat16


# ----------------------------------------------------------------------------
# device program
# ----------------------------------------------------------------------------

def build_program(M, cnt):
    """cnt[c] = number of global slots owned by core c (same on all cores;
    slot g of sorted clss lives at staging row c*M + rank-within-core)."""
    nc = bass.Bass()

    def inp(name, shape, dtype=F32):
        return nc.declare_dram_parameter(name, list(shape), dtype,
                                         isOutput=False)

    t = {}
    t["hT0ext"] = inp("hT0ext", [128, DT, E], BF16)
    t["h0tm"] = inp("h0tm", [T, D], BF16)
    t["hgT0"] = inp("hgT0", [128, DT, 64], BF16)
    for w in ("Wq", "Wk", "Wv", "Wqg", "Wkg", "Wvg", "Wo"):
        t[w] = inp(w, [L, D, D], BF16)
    t["Wf1"] = inp("Wf1", [L, D, FF], BF16)
    t["Wf2"] = inp("Wf2", [L, FF, D], BF16)
    for b in ("bq_p", "bk_p", "bkg_p", "bqg_p"):
        t[b] = inp(b, [L, 128, DT])
    t["bf1_p"] = inp("bf1_p", [L, 128, FT])
    for b in ("bv_b", "bvg_b", "bo_b", "bf2_b"):
        t[b] = inp(b, [L, 128, D], BF16)
    for b in ("ln1s_b", "ln1b_b", "ln2s_b", "ln2b_b"):
        t[b] = inp(b, [L, 128, D], BF16)
    t["stA_idx"] = inp("stA_idx", [128, 2], I32)
    t["stB_idx"] = inp("stB_idx", [128, 2], I32)
    t["exh_idx"] = inp("exh_idx", [128, 4], I32)
    t["gidx"] = inp("gidx", [M, 1], I32)
    t["grows"] = inp("grows", [M, 1], I32)
    t["gvinv"] = inp("gvinv", [M, 1])
    t["kval01"] = inp("kval01", [128, ET])
    t["gk01"] = inp("gk01", [64, 1])
    t["fk01"] = inp("fk01", [128, QT])
    t["glb1m"] = inp("glb1m", [128, QT])
    t["sel"] = inp("sel", [M, QT, 128], BF16)
    t["tri2"] = inp("tri2", [128, 256], BF16)
    t["trih2"] = inp("trih2", [128, 256], BF16)
    t["ident"] = inp("ident", [128, 128], BF16)
    t["ident8"] = inp("ident8", [128, 128], FP8)
    t["out"] = nc.declare_dram_parameter("out", [T, D], F32, isOutput=True)

    with tile.TileContext(nc) as tc:
        with (
            tc.tile_pool(name="cn", bufs=1) as cn,
            tc.tile_pool(name="wp", bufs=1) as wp,
            tc.tile_pool(name="act", bufs=1) as act,
            tc.tile_pool(name="scr", bufs=1) as scr,
            tc.tile_pool(name="pTp", bufs=1) as pTp,
            tc.tile_pool(name="psp", bufs=1, space="PSUM") as psp,
            tc.tile_pool(name="dram", bufs=1, space="DRAM") as dram,
        ):
            _body(nc, t, M, cnt, cn, wp, act, scr, pTp, psp, dram)
    return nc


def _body(nc, t, M, cnt, cn, wp, act, scr, pTp, psp, dram):
    base = [0] * (NC_ + 1)          # slot range [base[c], base[c+1]) per core
    for c in range(NC_):
        base[c + 1] = base[c] + cnt[c]
    assert base[NC_] == 64

    def load_const(name, shape, dtype=F32, eng=None):
        tl = cn.tile(list(shape), dtype, tag=name, name=name + "_sb")
        (eng or (nc.sync if NOGPD else nc.gpsimd)).dma_start(tl[:], t[name][:])
        return tl

    # first-needed activations ahead of everything on the Act queue
    hT_ext = act.tile([128, DT, E], BF16, tag="hT_ext", name="hT_ext")
    nc.scalar.dma_start(hT_ext[:], t["hT0ext"][:])

    tri2 = load_const("tri2", [128, 256], BF16)
    trih2 = load_const("trih2", [128, 256], BF16)
    ident = load_const("ident", [128, 128], BF16)
    ident8 = load_const("ident8", [128, 128], FP8)
    kval01 = load_const("kval01", [128, ET])
    gk01 = load_const("gk01", [64, 1], F32, nc.scalar)
    fk01 = load_const("fk01", [128, QT], F32, nc.scalar)
    glb1m = load_const("glb1m", [128, QT], F32, nc.scalar)
    stA_idx = load_const("stA_idx", [128, 2], I32)
    stB_idx = load_const("stB_idx", [128, 2], I32)
    exh_idx = load_const("exh_idx", [128, 4], I32)
    gidx = load_const("gidx", [M, 1], I32)
    grows = load_const("grows", [M, 1], I32)
    gvinv = load_const("gvinv", [M, 1])
    sel_sb = load_const("sel", [M, QT, 128], BF16)
    eps_c = cn.tile([128, 1], F32, tag="eps_c", name="eps_c")
    nc.vector.memset(eps_c[:], EPS)
    maxpad = max(max(M - c for c in cnt), 2)
    zero_pad = cn.tile([maxpad, 780], F32, tag="zpad", name="zpad")
    nc.vector.memset(zero_pad[:], 0.0)

    # hT_ext: h^T extended [left halo 256 | own 512 | right halo 256];
    # layer 0 comes from the host (loaded above), layer 1 is rebuilt in
    # place by PE transposes.
    hgT0 = cn.tile([128, DT, 64], BF16, tag="hgT0", name="hgT0")
    nc.scalar.dma_start(hgT0[:], t["hgT0"][:])
    h_bf = [act.tile([128, D], BF16, tag=f"hbf_{i}", name=f"hbf_{i}")
            for i in range(QT)]
    for i in range(QT):
        nc.scalar.dma_start(h_bf[i][:], t["h0tm"][i * 128:(i + 1) * 128, :])
    h1 = [act.tile([128, D], F32, tag=f"h1_{i}", name=f"h1_{i}")
          for i in range(QT)]

    h_loc = dram.tile([T, D], KD, tag="h_loc", name="h_loc")
    R = 256 + M               # per-rank rows in the round-B contribution

    def big32(name="b32"):
        return scr.tile([128, D], F32, tag="sD32", bufs=3, name=name)

    def small32(name="s32"):
        return scr.tile([128, 1], F32, tag="s1", bufs=32, name=name)

    def bigbf(name="bbf"):
        return scr.tile([128, D], BF16, tag="sDbf", bufs=1, name=name)

    # ---- batched layernorm, stage-major across tiles so tiny cross-engine
    # waits pipeline instead of serializing whole tiles. items: list of
    # (x_tile_f32, out_ap); out may alias x. var = E[x^2] - mean^2
    def layer_norm_batch(items, s_b, b_b):
        if NOLN:
            for x, out_ap in items:
                red = small32("ln_red")
                nc.vector.tensor_reduce(red[:], x[:], axis=AX.X, op=OP.add)
                mean = small32("ln_mean")
                nc.scalar.mul(mean[:], red[:], 1.0 / D)
                sq = big32("ln_sq")
                ssq = small32("ln_ssq")
                nc.scalar.activation(sq[:], x[:], AF.Square,
                                     accum_out=ssq[:, 0:1])
                vb = small32("ln_vb")
                nc.vector.tensor_scalar(vb[:], mean[:], mean[:, 0:1], -1.0,
                                        op0=OP.mult, op1=OP.mult)
                nc.vector.tensor_scalar_add(vb[:], vb[:], eps_c[:, 0:1])
                std = small32("ln_std")
                nc.scalar.activation(std[:], ssq[:], AF.Sqrt,
                                     bias=vb[:, 0:1], scale=1.0 / D)
                rstd = small32("ln_rstd")
                nc.vector.reciprocal(rstd[:], std[:])
                zn = big32("ln_zn")
                nc.vector.tensor_scalar(zn[:], x[:], mean[:, 0:1],
                                        rstd[:, 0:1], op0=OP.subtract,
                                        op1=OP.mult)
                tmp = big32("ln_tmp")
                nc.vector.tensor_mul(tmp[:], zn[:], s_b[:])
                nc.vector.tensor_add(out_ap, tmp[:], b_b[:])
            return
        n = len(items)
        mean = [small32(f"ln_mean{i}") for i in range(n)]
        ssq = [small32(f"ln_ssq{i}") for i in range(n)]
        vb = [small32(f"ln_vb{i}") for i in range(n)]
        std = [small32(f"ln_std{i}") for i in range(n)]
        rstd = [small32(f"ln_rstd{i}") for i in range(n)]
        red = [small32(f"ln_red{i}") for i in range(n)]
        for i, (x, _) in enumerate(items):
            nc.vector.tensor_reduce(red[i][:], x[:], axis=AX.X, op=OP.add)
            sq = big32(f"ln_sq{i}")
            nc.scalar.activation(sq[:], x[:], AF.Square,
                                 accum_out=ssq[i][:, 0:1])
        for i in range(n):
            nc.vector.tensor_scalar(mean[i][:], red[i][:], 1.0 / D, 0.0,
                                    op0=OP.mult, op1=OP.add)
        for i in range(n):
            nc.vector.tensor_scalar(vb[i][:], mean[i][:], mean[i][:, 0:1],
                                    -1.0, op0=OP.mult, op1=OP.mult)
            nc.vector.tensor_scalar_add(vb[i][:], vb[i][:], eps_c[:, 0:1])
        for i in range(n):
            nc.scalar.activation(std[i][:], ssq[i][:], AF.Sqrt,
                                 bias=vb[i][:, 0:1], scale=1.0 / D)
        for i in range(n):
            nc.vector.reciprocal(rstd[i][:], std[i][:])
        lne = nc.vector if NOPOOL else nc.gpsimd
        for i, (x, out_ap) in enumerate(items):
            lne.tensor_scalar(out_ap, x[:], mean[i][:, 0:1],
                              rstd[i][:, 0:1], op0=OP.subtract,
                              op1=OP.mult)
        for i, (_, out_ap) in enumerate(items):
            lne.tensor_mul(out_ap, out_ap, s_b[:])
        for i, (_, out_ap) in enumerate(items):
            lne.tensor_add(out_ap, out_ap, b_b[:])

    def wslab(src_ap, name, eng=None):
        tl = wp.tile([128, DT, D], BF16, tag="w", bufs=7, name=name)
        (eng or nc.sync).dma_start(
            tl[:], src_ap.rearrange("(k p) o -> p k o", p=128)[:])
        return tl

    def bload(l, name, n=DT, dtype=F32):
        tl = wp.tile([128, n], dtype, tag=f"b_{name}", bufs=2,
                     name=f"{name}{l}")
        nc.scalar.dma_start(tl[:], t[name][l][:])
        return tl

    # ---- feature-major projection; rhs read at rhs_off, out at col0.
    # use_act routes the psum bias-add through the Act engine.
    def proj_chunk(w_sb, b_sb, o, rhs_off, col0, w, use_act=False):
        for cc in range(0, w, 512):
            cw = min(cc + 512, w) - cc
            for ot in range(DT):
                ps = psp.tile([128, 512], F32, tag="p", bufs=8, name="pw")
                for k in range(DT):
                    nc.tensor.matmul(
                        ps[:, 0:cw],
                        w_sb[:, k, ot * 128:(ot + 1) * 128],
                        hT_ext[:, k, rhs_off + cc:rhs_off + cc + cw],
                        start=(k == 0), stop=(k == DT - 1))
                if use_act:
                    nc.scalar.activation(
                        o[:, ot, col0 + cc:col0 + cc + cw], ps[:, 0:cw],
                        AF.Identity, bias=b_sb[:, ot:ot + 1])
                else:
                    nc.vector.tensor_scalar_add(
                        o[:, ot, col0 + cc:col0 + cc + cw], ps[:, 0:cw],
                        b_sb[:, ot:ot + 1])

    # token-major v projection into [128, dst, H*65] layout w/ ones column;
    # pairs = [(dst_tile, src_ext_tile)]
    def proj_tm_tiles(w_sb, b_bc, o, pairs, use_act=False):
        for dst_tt, src_tt in pairs:
            for c0 in (0, 512):
                c1 = min(c0 + 512, D)
                ps = psp.tile([128, 512], F32, tag="p", bufs=8, name="pt")
                for k in range(DT):
                    nc.tensor.matmul(
                        ps[:, 0:c1 - c0],
                        hT_ext[:, k, src_tt * 128:(src_tt + 1) * 128],
                        w_sb[:, k, c0:c1],
                        start=(k == 0), stop=(k == DT - 1))
                biased = big32("ptb")
                if use_act:
                    nc.scalar.activation(biased[:, 0:c1 - c0],
                                         ps[:, 0:c1 - c0], AF.Identity)
                    src = biased
                    nc.gpsimd.tensor_add(biased[:, 0:c1 - c0],
                                         biased[:, 0:c1 - c0], b_bc[:, c0:c1])
                else:
                    nc.vector.tensor_add(biased[:, 0:c1 - c0],
                                         ps[:, 0:c1 - c0], b_bc[:, c0:c1])
                nh = (c1 - c0) // 64
                h0_ = c0 // 64
                dst = o[:, dst_tt].rearrange("p (hh c) -> p hh c", c=65)
                (nc.gpsimd if use_act else nc.vector).tensor_copy(
                    dst[:, h0_:h0_ + nh, 0:64],
                    biased[:, 0:c1 - c0]
                    .rearrange("p (hh c) -> p hh c", c=64)[:])
            (nc.gpsimd if use_act else nc.vector).memset(
                o[:, dst_tt].rearrange("p (hh c) -> p hh c", c=65)
                [:, :, 64:65], 1.0)

    def proj_fm_g(l, w_sb, b_sb, hgT, tag):
        o = act.tile([128, DT, 64], BF16, tag=tag, name=tag + str(l))
        for ot in range(DT):
            ps = psp.tile([128, 512], F32, tag="p", bufs=8, name="pg_")
            for k in range(DT):
                nc.tensor.matmul(
                    ps[:, 0:64], w_sb[:, k, ot * 128:(ot + 1) * 128],
                    hgT[:, k, :],
                    start=(k == 0), stop=(k == DT - 1))
            nc.vector.tensor_scalar_add(o[:, ot, :], ps[:, 0:64],
                                        b_sb[:, ot:ot + 1])
        return o

    def pe_transpose(src_ap, dst_ap, ceng, rows=128, idt=None):
        tdt = BF16 if idt is None else FP8
        idt = ident if idt is None else idt
        tps = psp.tile([128, 128], tdt, tag="p", bufs=8, name="tr")
        if rows < 128:
            nc.tensor.transpose(tps[:, 0:rows], src_ap,
                                idt[0:rows, 0:rows])
            ceng.tensor_copy(dst_ap, tps[:, 0:rows])
        else:
            nc.tensor.transpose(tps[:], src_ap, idt[:])
            ceng.tensor_copy(dst_ap, tps[:])

    # ---------------- layers ----------------
    for l in range(LL):
        # ---- proj weight slabs (SP), loaded in approximate death order so
        # the FIFO buffer rotation evicts dead slabs first
        w_q = wslab(t["Wq"][l], f"wq{l}")
        w_kg = wslab(t["Wkg"][l], f"wkg{l}")
        w_vg = wslab(t["Wvg"][l], f"wvg{l}")
        w_qg = wslab(t["Wqg"][l], f"wqg{l}")
        w_k = wslab(t["Wk"][l], f"wk{l}")
        w_v = wslab(t["Wv"][l], f"wv{l}")
        w_o = wslab(t["Wo"][l], f"wo{l}")
        b_q = bload(l, "bq_p")
        b_k = bload(l, "bk_p")
        b_kg = bload(l, "bkg_p")
        b_qg = bload(l, "bqg_p")
        b_f1 = bload(l, "bf1_p", FT)
        b_v = bload(l, "bv_b", D, BF16)
        b_vg = bload(l, "bvg_b", D, BF16)
        b_o = bload(l, "bo_b", D, BF16)
        b_f2 = bload(l, "bf2_b", D, BF16)
        ln1s = bload(l, "ln1s_b", D, BF16)
        ln1b = bload(l, "ln1b_b", D, BF16)
        ln2s = bload(l, "ln2s_b", D, BF16)
        ln2b = bload(l, "ln2b_b", D, BF16)

        # ---- layer-1 staging + collectives (Pool queue head of layer).
        # Round A: pairwise exchange of the A-parity 256-row boundary chunks.
        # Round B: all-8 AllGather of the B-parity chunks with each core's M
        # owned global-h rows appended (fp8 throughout; h_loc is fp8).
        if l > 0:
            cbA = dram.tile([256, D], KD, tag="cbA", name="cbA")
            cbB = dram.tile([R, D], KD, tag="cbB", name="cbB")
            outAB = dram.tile([512 + NC_ * R, D], KD, tag="outAB",
                              name="outAB")
            for idxt, cb, nm in ((stB_idx, cbB, "B"), (stA_idx, cbA, "A")):
                for j in range(2):
                    st_sb = scr.tile([128, D], KD, tag="sDf8", bufs=(2 if KD == BF16 else 4),
                                     name=f"st{nm}{j}")
                    nc.gpsimd.indirect_dma_start(
                        out=st_sb[:], out_offset=None, in_=h_loc[:],
                        in_offset=bass.IndirectOffsetOnAxis(
                            ap=idxt[:, j:j + 1], axis=0))
                    nc.sync.dma_start(cb[j * 128:(j + 1) * 128, :], st_sb[:])
                if nm == "B":
                    cg_sb = scr.tile([M, D], KD, tag="cg_sb", name="cg_sb")
                    nc.gpsimd.indirect_dma_start(
                        out=cg_sb[:], out_offset=None, in_=h_loc[:],
                        in_offset=bass.IndirectOffsetOnAxis(ap=gidx[:, 0:1],
                                                            axis=0))
                    nc.sync.dma_start(cbB[256:256 + M, :], cg_sb[:])
            nc.gpsimd.collective_compute(
                "AllGather", OP.bypass, ins=[cbB[:]],
                outs=[outAB[512:512 + NC_ * R]],
                replica_groups=GRP_ALL)
            nc.gpsimd.collective_compute(
                "AllGather", OP.bypass, ins=[cbA[:]], outs=[outAB[0:512]],
                replica_groups=GRP_A)

            # ---- hT_ext own columns via PE transposes (DVE copies)
            for i in range(QT):
                for d in range(DT):
                    pe_transpose(
                        h_bf[i][:, d * 128:(d + 1) * 128],
                        hT_ext[:, d, 256 + i * 128:256 + (i + 1) * 128],
                        nc.vector)

        # ---- local projections (own columns of hT_ext)
        # Pool cannot touch PSUM on real hw; psum consumers go to DVE or
        # Act (Identity activation + bias) -- Act is idle in layer 0's
        # projection phase.
        ua = (l == 0) and not NOPOOL
        qT = act.tile([128, DT, T], BF16, tag="qT", name=f"qT{l}")
        proj_chunk(w_q, b_q, qT, 256, 0, T, use_act=ua)
        kgfT = act.tile([128, DT, T], BF16, tag="kgfT", name=f"kgfT{l}")
        proj_chunk(w_kg, b_kg, kgfT, 256, 0, T, use_act=ua)
        kT = act.tile([128, DT, E], BF16, tag="kT", name=f"kT{l}")
        v_sb = act.tile([128, ET, H * 65], BF16, tag="v_sb", name=f"v{l}")
        vgf_sb = act.tile([128, QT, H * 65], BF16, tag="vgf_sb",
                          name=f"vgf{l}")
        if l == 0:
            proj_chunk(w_k, b_k, kT, 0, 0, E, use_act=ua)
            proj_tm_tiles(w_v, b_v, v_sb, [(tt, tt) for tt in range(ET)],
                          use_act=ua)
        else:
            proj_chunk(w_k, b_k, kT, 256, 256, T)
            proj_tm_tiles(w_v, b_v, v_sb, [(tt, tt) for tt in (2, 3, 4, 5)])
        proj_tm_tiles(w_vg, b_vg, vgf_sb,
                      [(qt, qt + 2) for qt in range(QT)], use_act=ua)
        veng = nc.vector if NOPOOL else (nc.gpsimd if l == 0 else nc.vector)
        for kt in range(QT):
            veng.tensor_scalar_mul(vgf_sb[:, kt, :], vgf_sb[:, kt, :],
                                   fk01[:, kt:kt + 1])

        # ---- global rows -> hgT (layer 1: plain DMAs from gg, owner-order)
        if l == 0:
            hgT = hgT0
        else:
            hg_tm8 = scr.tile([64, D], KD, tag="hg_tm8", name="hg_tm8")
            for c in range(NC_):
                if cnt[c]:
                    nc.scalar.dma_start(
                        hg_tm8[base[c]:base[c + 1], :],
                        outAB[512 + c * R + 256:512 + c * R + 256 + cnt[c],
                              :])
            hg_tm = scr.tile([64, D], BF16, tag="hg_tm", name="hg_tm")
            nc.vector.tensor_copy(hg_tm[:], hg_tm8[:])
            hgT = act.tile([128, DT, 64], BF16, tag="hgT", name=f"hgT{l}")
            for d in range(DT):
                pe_transpose(hg_tm[0:64, d * 128:(d + 1) * 128],
                             hgT[:, d, :], nc.vector, rows=64)

        qgT = proj_fm_g(l, w_qg, b_qg, hgT, "qgT")
        kgT = proj_fm_g(l, w_k, b_k, hgT, "kgT")

        vg_sb = act.tile([64, H * 65], BF16, tag="vg_sb", name=f"vg{l}")
        for c0 in (0, 512):
            c1 = min(c0 + 512, D)
            ps = psp.tile([128, 512], F32, tag="p", bufs=8, name="pvg")
            for k in range(DT):
                nc.tensor.matmul(ps[0:64, 0:c1 - c0], hgT[:, k, :],
                                 w_v[:, k, c0:c1],
                                 start=(k == 0), stop=(k == DT - 1))
            biased = big32("vgb")
            nc.vector.tensor_add(biased[0:64, 0:c1 - c0], ps[0:64, 0:c1 - c0],
                                 b_v[0:64, c0:c1])
            nh = (c1 - c0) // 64
            h0_ = c0 // 64
            dst = vg_sb.rearrange("p (hh c) -> p hh c", c=65)
            nc.vector.tensor_copy(
                dst[:, h0_:h0_ + nh, 0:64],
                biased[0:64, 0:c1 - c0]
                .rearrange("p (hh c) -> p hh c", c=64)[:])
        nc.vector.memset(
            vg_sb.rearrange("p (hh c) -> p hh c", c=65)[:, :, 64:65], 1.0)
        # invalid global keys: zero the whole row (removes them from both
        # numerator and denominator, replacing the old exp-bias masking)
        nc.vector.tensor_scalar_mul(vg_sb[:], vg_sb[:], gk01[:, 0:1])

        # ---- global-query attention partials -> stag
        stage = dram.tile([NC_ * M, 780], F32, tag="stage", name=f"stage{l}")
        rs_out = dram.tile([M, 780], F32, tag="rs_out", name=f"rso{l}")
        stag = scr.tile([64, H, 65], F32, tag="gq_stage", name=f"stag{l}")
        for hh in range(H):
            hp, hr = hh // 2, (hh % 2) * 64
            prow = slice(hr, hr + 64)
            sps = psp.tile([128, 256], F32, tag="p", bufs=8, name="sf")
            for kt in range(QT):
                nc.tensor.matmul(
                    sps[:, kt * 64:(kt + 1) * 64],
                    kgfT[prow, hp, kt * 128:(kt + 1) * 128],
                    qgT[prow, hp, :], start=True, stop=True)
            pf = pTp.tile([128, 256], BF16, tag="pf", bufs=3, name="pf")
            nc.scalar.activation(pf[:], sps[:], AF.Exp, scale=SCALE)
            gps = psp.tile([128, 65], F32, tag="p", bufs=8, name="gps")
            for kt in range(QT):
                nc.tensor.matmul(gps[0:64, :],
                                 pf[:, kt * 64:(kt + 1) * 64],
                                 vgf_sb[:, kt, hh * 65:(hh + 1) * 65],
                                 start=(kt == 0), stop=(kt == QT - 1))
            nc.vector.tensor_copy(stag[:, hh, :], gps[0:64, :])
        # owner-order staging via plain DMAs (clss sorted -> slots per core
        # are a contiguous stag range); pads zeroed from a zero tile
        for c in range(NC_):
            seng = nc.sync if NOGPD else nc.gpsimd
            if cnt[c]:
                seng.dma_start(
                    stage[c * M:c * M + cnt[c], :],
                    stag.rearrange("p a b -> p (a b)")[base[c]:base[c + 1]])
            if cnt[c] < M:
                seng.dma_start(stage[c * M + cnt[c]:(c + 1) * M, :],
                               zero_pad[0:M - cnt[c], :])

        # ---- layer-1 halo extraction (after round B)
        if l > 0:
            for g in range(4):  # 0,1 left; 2,3 right
                htmp8 = scr.tile([128, D], KD, tag="sDf8", bufs=(2 if KD == BF16 else 4),
                                 name=f"halo{g}")
                nc.gpsimd.indirect_dma_start(
                    out=htmp8[:], out_offset=None, in_=outAB[:],
                    in_offset=bass.IndirectOffsetOnAxis(
                        ap=exh_idx[:, g:g + 1], axis=0))
                htmp = bigbf(f"halobf{g}")
                nc.vector.tensor_copy(htmp[:], htmp8[:])
                off = g * 128 if g < 2 else 768 + (g - 2) * 128
                for d in range(DT):
                    pe_transpose(htmp[:, d * 128:(d + 1) * 128],
                                 hT_ext[:, d, off:off + 128], nc.vector)

        if NORS:
            ar_out = dram.tile([NC_ * M, 780], F32, tag="ar_out",
                               name=f"aro{l}", addr_space="Shared")
            nc.gpsimd.collective_compute(
                "AllReduce", OP.add, ins=[stage[:]], outs=[ar_out[:]],
                replica_groups=GRP_ALL)
        else:
            nc.gpsimd.collective_compute(
                "ReduceScatter", OP.add, ins=[stage[:]], outs=[rs_out[:]],
                replica_groups=GRP_ALL)

        # ---- FFN slabs on Pool, after the RS so they never delay it; the
        # two half-1 f2 slabs come after LN1 (their WAR frees late)
        fw = [None] * 8
        for half in range(2):
            fw[half * 4 + 0] = wslab(
                t["Wf1"][l][:, half * 1536:half * 1536 + 768],
                f"f1a{l}{half}", eng=(nc.scalar if NOGPD else nc.gpsimd))
            fw[half * 4 + 1] = wslab(
                t["Wf1"][l][:, half * 1536 + 768:(half + 1) * 1536],
                f"f1b{l}{half}", eng=(nc.scalar if NOGPD else nc.gpsimd))
            if half == 0:
                fw[2] = wslab(t["Wf2"][l][0:768, :], f"f2a{l}0",
                              eng=(nc.scalar if NOGPD else nc.gpsimd))
                fw[3] = wslab(t["Wf2"][l][768:1536, :], f"f2b{l}0",
                              eng=(nc.scalar if NOGPD else nc.gpsimd))

        # ---- layer-1 halo projections
        if l > 0:
            proj_chunk(w_k, b_k, kT, 0, 0, 256)
            proj_chunk(w_k, b_k, kT, 768, 768, 256)
            proj_tm_tiles(w_v, b_v, v_sb, [(tt, tt) for tt in (0, 1, 6, 7)])

        # fold key-validity masking into v (zero rows of invalid keys)
        for e in range(ET):
            nc.vector.tensor_scalar_mul(v_sb[:, e, :], v_sb[:, e, :],
                                        kval01[:, e:e + 1])

        # ---- band + global-key attention -> o_sb; head-pair-major so a
        # pair's global-key probs (one exp over all queries) stay resident
        o_sb = act.tile([128, QT, D], BF16, tag="o_sb", name=f"osb{l}")
        if NOBAND:
            for qt in range(QT):
                qsl = slice(qt * 128, (qt + 1) * 128)
                for hh in range(H):
                    hp, hr = hh // 2, (hh % 2) * 64
                    prow = slice(hr, hr + 64)
                    sp4 = psp.tile([128, 512], F32, tag="p", bufs=8,
                                   name="sp4")
                    for a in range(4):
                        e = qt + a
                        nc.tensor.matmul(
                            sp4[:, a * 128:(a + 1) * 128],
                            kT[prow, hp, e * 128:(e + 1) * 128],
                            qT[prow, hp, qsl], start=True, stop=True)
                    sp_hi = psp.tile([128, 128], F32, tag="p", bufs=8,
                                     name="sp_hi")
                    nc.tensor.matmul(
                        sp_hi[:],
                        kT[prow, hp, (qt + 4) * 128:(qt + 5) * 128],
                        qT[prow, hp, qsl], start=True, stop=True)
                    pt = pTp.tile([128, 640], BF16, tag="ptb6", bufs=2,
                                  name="pt")
                    nc.scalar.activation(pt[:, 128:640], sp4[:], AF.Exp,
                                         scale=SCALE)
                    nc.scalar.activation(pt[:, 0:128], sp_hi[:], AF.Exp,
                                         scale=SCALE)
                    nc.vector.tensor_mul(pt[:, 0:256], pt[:, 0:256],
                                         tri2[:])
                    sg = psp.tile([128, 128], F32, tag="p", bufs=8,
                                  name="sg")
                    nc.tensor.matmul(sg[0:64, :], kgT[prow, hp, :],
                                     qT[prow, hp, qsl],
                                     start=True, stop=True)
                    pg = pTp.tile([64, 128], BF16, tag="pgb", bufs=2,
                                  name="pg")
                    nc.scalar.activation(pg[:], sg[0:64, :], AF.Exp,
                                         scale=SCALE)
                    ops = psp.tile([128, 65], F32, tag="p", bufs=8,
                                   name="ops")
                    for a in range(4):
                        e = qt + a
                        nc.tensor.matmul(
                            ops[:], pt[:, 128 + a * 128:256 + a * 128],
                            v_sb[:, e, hh * 65:(hh + 1) * 65],
                            start=(a == 0), stop=False)
                    nc.tensor.matmul(
                        ops[:], pt[:, 0:128],
                        v_sb[:, qt + 4, hh * 65:(hh + 1) * 65],
                        start=False, stop=False)
                    nc.tensor.matmul(ops[:], pg[:],
                                     vg_sb[:, hh * 65:(hh + 1) * 65],
                                     start=False, stop=True)
                    rec = small32("rec")
                    nc.vector.reciprocal(rec[:], ops[:, 64:65])
                    fac = small32("fac")
                    nc.vector.tensor_mul(fac[:], rec[:],
                                         glb1m[:, qt:qt + 1])
                    nc.vector.tensor_scalar_mul(
                        o_sb[:, qt, hh * 64:(hh + 1) * 64], ops[:, 0:64],
                        fac[:, 0:1])
        for hp in range(H // 2) if not NOBAND else []:
            pg_pair = []
            if not NOSG:
                for s in range(2):
                    hh = 2 * hp + s
                    prow = slice(s * 64, s * 64 + 64)
                    sg = psp.tile([128, 512], F32, tag="p", bufs=8, name="sg")
                    nc.tensor.matmul(sg[0:64, :], kgT[prow, hp, :],
                                     qT[prow, hp, :], start=True, stop=True)
                    pg = pTp.tile([64, 512], BF16, tag="pg",
                                  bufs=(1 if KD == BF16 else 2), name="pg")
                    nc.scalar.activation(pg[:], sg[0:64, :], AF.Exp,
                                         scale=SCALE)
                    pg_pair.append(pg)
            for qt in range(QT):
                qsl = slice(qt * 128, (qt + 1) * 128)
                pts = []
                # hi tiles of both heads in one psum -> one exp
                if NOHI:
                    pth = pTp.tile([128, 256], BF16, tag="pf", bufs=3,
                                   name="pth")
                    for s in range(2):
                        prow = slice(s * 64, s * 64 + 64)
                        sph1 = psp.tile([128, 128], F32, tag="p", bufs=8,
                                        name="sph1")
                        nc.tensor.matmul(
                            sph1[:],
                            kT[prow, hp, (qt + 4) * 128:(qt + 5) * 128],
                            qT[prow, hp, qsl], start=True, stop=True)
                        nc.scalar.activation(pth[:, s * 128:(s + 1) * 128],
                                             sph1[:], AF.Exp, scale=SCALE)
                    nc.vector.tensor_mul(pth[:], pth[:], trih2[:])
                else:
                    sp_hi = psp.tile([128, 256], F32, tag="p", bufs=8,
                                     name="sp_hi")
                    for s in range(2):
                        prow = slice(s * 64, s * 64 + 64)
                        nc.tensor.matmul(
                            sp_hi[:, s * 128:(s + 1) * 128],
                            kT[prow, hp, (qt + 4) * 128:(qt + 5) * 128],
                            qT[prow, hp, qsl], start=True, stop=True)
                for s in range(2):
                    prow = slice(s * 64, s * 64 + 64)
                    sp4 = psp.tile([128, 512], F32, tag="p", bufs=8,
                                   name="sp4")
                    for a in range(4):
                        e = qt + a
                        nc.tensor.matmul(
                            sp4[:, a * 128:(a + 1) * 128],
                            kT[prow, hp, e * 128:(e + 1) * 128],
                            qT[prow, hp, qsl], start=True, stop=True)
                    pt = pTp.tile([128, 512], BF16, tag="pt", bufs=(2 if KD == BF16 else 3),
                                  name="pt")
                    nc.scalar.activation(pt[:], sp4[:], AF.Exp, scale=SCALE)
                    nc.vector.tensor_mul(pt[:, 0:128], pt[:, 0:128],
                                         tri2[:, 128:256])
                    pts.append(pt)
                if not NOHI:
                    pth = pTp.tile([128, 256], BF16, tag="pf", bufs=3,
                                   name="pth")
                    nc.scalar.activation(pth[:], sp_hi[:], AF.Exp,
                                         scale=SCALE)
                    nc.vector.tensor_mul(pth[:], pth[:], trih2[:])
                for s in range(2):
                    hh = 2 * hp + s
                    ops = psp.tile([128, 65], F32, tag="p", bufs=8,
                                   name="ops")
                    for a in range(4):
                        e = qt + a
                        nc.tensor.matmul(
                            ops[:], pts[s][:, a * 128:(a + 1) * 128],
                            v_sb[:, e, hh * 65:(hh + 1) * 65],
                            start=(a == 0), stop=False)
                    nc.tensor.matmul(
                        ops[:], pth[:, s * 128:(s + 1) * 128],
                        v_sb[:, qt + 4, hh * 65:(hh + 1) * 65],
                        start=False, stop=NOSG)
                    if not NOSG:
                        nc.tensor.matmul(
                            ops[:], pg_pair[s][:, qsl],
                            vg_sb[:, hh * 65:(hh + 1) * 65],
                            start=False, stop=True)
                    rec = small32("rec")
                    nc.vector.reciprocal(rec[:], ops[:, 64:65])
                    fac = small32("fac")
                    nc.vector.tensor_mul(fac[:], rec[:], glb1m[:, qt:qt + 1])
                    nc.vector.tensor_scalar_mul(
                        o_sb[:, qt, hh * 64:(hh + 1) * 64], ops[:, 0:64],
                        fac[:, 0:1])

        # ---- og from the ReduceScatter result; scatter into o_sb
        gsum = scr.tile([M, H, 65], F32, tag="gq_sum", name=f"gsum{l}")
        if NORS:
            nc.gpsimd.indirect_dma_start(
                out=gsum.rearrange("p a b -> p (a b)")[:], out_offset=None,
                in_=ar_out[:],
                in_offset=bass.IndirectOffsetOnAxis(ap=grows[:, 0:1], axis=0))
        else:
            nc.scalar.dma_start(gsum.rearrange("p a b -> p (a b)")[:],
                                rs_out[:])
        dens = scr.tile([M, H], F32, tag="dens", name=f"dens{l}")
        nc.vector.tensor_scalar_add(
            dens[:], gsum[:, :, 64:65].rearrange("p a b -> p (a b)")[:],
            gvinv[:, 0:1])
        recg = scr.tile([M, H], F32, tag="recg", name=f"recg{l}")
        nc.vector.reciprocal(recg[:], dens[:])
        og = act.tile([M, D], BF16, tag="og", name=f"og{l}")
        for hh in range(H):
            nc.vector.tensor_scalar_mul(og[:, hh * 64:(hh + 1) * 64],
                                        gsum[:, hh, 0:64],
                                        recg[:, hh:hh + 1])
        for qt in range(QT):
            for c0 in (0, 512):
                c1 = min(c0 + 512, D)
                sc = psp.tile([128, 512], F32, tag="p", bufs=8, name="sc")
                nc.tensor.matmul(sc[:, 0:c1 - c0], sel_sb[:, qt, :],
                                 og[:, c0:c1], start=True, stop=True)
                nc.vector.tensor_add(o_sb[:, qt, c0:c1], o_sb[:, qt, c0:c1],
                                     sc[:, 0:c1 - c0])

        # ---- oT via PE transposes (shares qT slot; DVE copies)
        oT = act.tile([128, DT, T], BF16, tag="qT", name=f"oT{l}")
        for qt in range(QT):
            for d in range(DT):
                pe_transpose(o_sb[:, qt, d * 128:(d + 1) * 128],
                             oT[:, d, qt * 128:(qt + 1) * 128], nc.vector)

        # ---- Wo + residual accumulated in h1, then LN1 in place
        for qt in range(QT):
            nc.vector.tensor_add(h1[qt][:], h_bf[qt][:], b_o[:])
            for c0 in (0, 512):
                c1 = min(c0 + 512, D)
                ps = psp.tile([128, 512], F32, tag="p", bufs=8, name="pwo")
                for k in range(DT):
                    nc.tensor.matmul(
                        ps[:, 0:c1 - c0], oT[:, k, qt * 128:(qt + 1) * 128],
                        w_o[:, k, c0:c1],
                        start=(k == 0), stop=(k == DT - 1))
                nc.vector.tensor_add(h1[qt][:, c0:c1], h1[qt][:, c0:c1],
                                     ps[:, 0:c1 - c0])
        layer_norm_batch([(h1[qt], h1[qt][:]) for qt in range(QT)],
                         ln1s, ln1b)

        # ---- remaining FFN slabs (WAR on w_o frees at Wo above)
        fw[6] = wslab(t["Wf2"][l][1536:2304, :], f"f2a{l}1", eng=(nc.scalar if NOGPD else nc.gpsimd))
        fw[7] = wslab(t["Wf2"][l][2304:3072, :], f"f2b{l}1", eng=(nc.scalar if NOGPD else nc.gpsimd))

        # ---- h1T via PE transposes (shares kgfT slot; DVE copies)
        h1T = act.tile([128, DT, T], BF16, tag="kgfT", name=f"h1T{l}")
        for qt in range(QT):
            h1b = bigbf(f"h1b{qt}")
            nc.vector.tensor_copy(h1b[:], h1[qt][:])
            for d in range(DT):
                pe_transpose(h1b[:, d * 128:(d + 1) * 128],
                             h1T[:, d, qt * 128:(qt + 1) * 128], nc.vector)

        # ---- FFN: x2 accumulates in-place on h1 (f32)
        for qt in range(QT):
            nc.vector.tensor_add(h1[qt][:], h1[qt][:], b_f2[:])
        for half in range(2):
            f1a, f1b, f2a, f2b = fw[half * 4:half * 4 + 4]
            gT = act.tile([128, FT // 2, T], BF16, tag="v_sb", bufs=1,
                          name=f"gT{l}{half}")
            for ft in range(FT // 2):
                fabs = half * (FT // 2) + ft
                slab = f1a if ft < 6 else f1b
                ps = psp.tile([128, 512], F32, tag="p", bufs=8, name="pf1")
                for k in range(DT):
                    nc.tensor.matmul(
                        ps[:], slab[:, k, (ft % 6) * 128:(ft % 6 + 1) * 128],
                        h1T[:, k, :],
                        start=(k == 0), stop=(k == DT - 1))
                nc.scalar.activation(gT[:, ft, :], ps[:], AF.Gelu_apprx_tanh,
                                     bias=b_f1[:, fabs:fabs + 1])
            for qt in range(QT):
                for c0 in (0, 512):
                    c1 = min(c0 + 512, D)
                    ps = psp.tile([128, 512], F32, tag="p", bufs=8,
                                  name="pf2")
                    for k in range(FT // 2):
                        slab = f2a if k < 6 else f2b
                        nc.tensor.matmul(
                            ps[:, 0:c1 - c0],
                            gT[:, k, qt * 128:(qt + 1) * 128],
                            slab[:, k % 6, c0:c1],
                            start=(k == 0), stop=(k == FT // 2 - 1))
                    nc.vector.tensor_add(h1[qt][:, c0:c1], h1[qt][:, c0:c1],
                                         ps[:, 0:c1 - c0])
        if l + 1 < L:
            layer_norm_batch([(h1[qt], h_bf[qt][:]) for qt in range(QT)],
                             ln2s, ln2b)
            if LL == 1:
                for qt in range(QT):
                    nc.sync.dma_start(t["out"][qt * 128:(qt + 1) * 128, :],
                                      h1[qt][:])
            for qt in range(QT):
                h8 = scr.tile([128, D], KD, tag="sDf8", bufs=(2 if KD == BF16 else 4),
                              name=f"h8_{qt}")
                (nc.vector if NOPOOL else nc.gpsimd).tensor_copy(h8[:], h_bf[qt][:])
                nc.sync.dma_start(h_loc[qt * 128:(qt + 1) * 128, :], h8[:])
        else:
            layer_norm_batch([(h1[qt], h1[qt][:]) for qt in range(QT)],
                             ln2s, ln2b)
            for qt in range(QT):
                nc.sync.dma_start(t["out"][qt * 128:(qt + 1) * 128, :],
                                  h1[qt][:])


# ----------------------------------------------------------------------------
# host side
# ----------------------------------------------------------------------------

_prog_cache = {}


def _get_program(M, cnt):
    key = (M,) + tuple(cnt)
    if key not in _prog_cache:
        _prog_cache[key] = build_program(M, list(cnt))
    return _prog_cache[key]


def _prep_maps(inputs):
    gi = {k: np.asarray(v) for k, v in inputs.items()}
    x = gi["x"][0]
    segs = gi["segs"][0]
    mask = gi["mask_src"][0] > 0
    clss = gi["clss"][0]
    assert np.all(np.diff(clss) >= 0), "clss must be sorted"

    is_glb = np.zeros(S, bool)
    is_glb[clss] = True

    # ---- h0 = LN(word + pos + type) on host, f32 -> bf16
    emb = (gi["word_emb"][x] + gi["pos_emb"][np.arange(S)]
           + gi["type_emb"][segs]).astype(np.float32)
    mu = emb.mean(-1, keepdims=True)
    var = ((emb - mu) ** 2).mean(-1, keepdims=True)
    h0 = ((emb - mu) / np.sqrt(var + EPS) * gi["ln_e_s"]
          + gi["ln_e_b"]).astype(np.float32)

    # owner-order bookkeeping for the 64 global slots (clss sorted => slots
    # owned by core c form the contiguous range [base[c], base[c+1]))
    owner = clss // T
    cnt = np.bincount(owner, minlength=NC_)
    M = max(int(cnt.max()), 64 // NC_)
    base = np.concatenate([[0], np.cumsum(cnt)])

    # scatter representative: one entry per position (duplicates collapse)
    rep = np.zeros(64, bool)
    seen = set()
    for g in range(63, -1, -1):
        if int(clss[g]) not in seen:
            seen.add(int(clss[g]))
            rep[g] = True

    def bcast(v, dt=np.float32):
        v = np.asarray(v, np.float32)
        return np.broadcast_to(v[None, :], (128, v.shape[0])).astype(dt)

    def part(v):
        return np.asarray(v, np.float32).reshape(-1, 128).T.copy()

    def fmaj(rows):  # [n, D] -> [128, DT, n]
        return np.ascontiguousarray(
            rows.T.reshape(DT, 128, rows.shape[0]).transpose(1, 0, 2))

    tri_lo = (np.arange(128)[:, None] >= np.arange(128)[None, :])
    tri_hi = (np.arange(128)[:, None] <= np.arange(128)[None, :])

    shared = {
        "Wq": gi["Wq"].astype(bfd), "Wk": gi["Wk"].astype(bfd),
        "Wv": gi["Wv"].astype(bfd), "Wqg": gi["Wqg"].astype(bfd),
        "Wkg": gi["Wkg"].astype(bfd), "Wvg": gi["Wvg"].astype(bfd),
        "Wo": gi["Wo"].astype(bfd),
        "Wf1": gi["Wf1"].astype(bfd), "Wf2": gi["Wf2"].astype(bfd),
        "bq_p": np.stack([part(gi["bq"][l]) for l in range(L)]),
        "bk_p": np.stack([part(gi["bk"][l]) for l in range(L)]),
        "bkg_p": np.stack([part(gi["bkg"][l]) for l in range(L)]),
        "bqg_p": np.stack([part(gi["bqg"][l]) for l in range(L)]),
        "bf1_p": np.stack([part(gi["bf1"][l]) for l in range(L)]),
        "bv_b": np.stack([bcast(gi["bv"][l], bfd) for l in range(L)]),
        "bvg_b": np.stack([bcast(gi["bvg"][l], bfd) for l in range(L)]),
        "bo_b": np.stack([bcast(gi["bo"][l], bfd) for l in range(L)]),
        "bf2_b": np.stack([bcast(gi["bf2"][l], bfd) for l in range(L)]),
        "ln1s_b": np.stack([bcast(gi["ln1_s"][l], bfd) for l in range(L)]),
        "ln1b_b": np.stack([bcast(gi["ln1_b"][l], bfd) for l in range(L)]),
        "ln2s_b": np.stack([bcast(gi["ln2_s"][l], bfd) for l in range(L)]),
        "ln2b_b": np.stack([bcast(gi["ln2_b"][l], bfd) for l in range(L)]),
        "hgT0": fmaj(h0[clss]).astype(bfd),
        "gk01": mask[clss].astype(np.float32).reshape(64, 1),
        "tri2": np.concatenate([tri_hi, tri_lo], axis=1).astype(bfd),
        "trih2": np.concatenate([tri_hi, tri_hi], axis=1).astype(bfd),
        "ident": np.eye(128, dtype=bfd),
        "ident8": np.eye(128, dtype=ml_dtypes.float8_e4m3),
    }

    maps = []
    for c in range(NC_):
        s0, s1 = c * T, (c + 1) * T
        toks = np.arange(s0, s1)
        ext = np.arange(s0 - 256, s1 + 256)
        ext_ok = (ext >= 0) & (ext < S)
        extc = np.clip(ext, 0, S - 1)
        kval = ext_ok & mask[extc] & ~is_glb[extc]
        h0ext = np.where(ext_ok[:, None], h0[extc], 0.0).astype(np.float32)

        # round A (pairs): even cores contribute last 256 rows, odd first.
        # round B (all-8): even first 256, odd last 256, + M global rows.
        lastr = np.arange(256, 512, dtype=np.int32)
        firstr = np.arange(0, 256, dtype=np.int32)
        stA = lastr if c % 2 == 0 else firstr
        stB = firstr if c % 2 == 0 else lastr
        # extraction rows in outAB: [0:512] = my A-pair out (rank order),
        # [512 + r*R : ...] = round-B contribution of core r. Invalid sides
        # (core 0 left, core 7 right) point at written rows; kval01 masks.
        R = 256 + M
        if c % 2 == 0:
            lb = 512 + (c - 1) * R if c > 0 else 512
            left = lb + firstr             # odd neighbor's last 256 (B)
            right = 256 + firstr           # A out rank-1 = odd's first 256
        else:
            left = firstr                  # A out rank-0 = even's last 256
            rb = 512 + (c + 1) * R if c < NC_ - 1 else 512 + c * R
            right = rb + firstr            # even neighbor's first 256 (B)
        exh = np.stack([left[:128], left[128:], right[:128], right[128:]],
                       axis=1).astype(np.int32)

        own = [int(g) for g in range(base[c], base[c + 1])]
        gidx_v = np.zeros((M, 1), np.int32)
        gv = np.zeros((M, 1), np.float32)
        sel = np.zeros((M, QT, 128), np.float32)
        for j, g in enumerate(own):
            p = int(clss[g])
            gidx_v[j, 0] = p - s0
            gv[j, 0] = 1.0
            if rep[g]:
                sel[j, (p - s0) // 128, (p - s0) % 128] = 1.0

        m = {
            "grows": (c * M + np.arange(M, dtype=np.int32)).reshape(M, 1),
            "hT0ext": fmaj(h0ext).astype(bfd),
            "h0tm": h0[s0:s1].astype(bfd),
            "stA_idx": np.stack([stA[:128], stA[128:]], axis=1),
            "stB_idx": np.stack([stB[:128], stB[128:]], axis=1),
            "exh_idx": exh,
            "gidx": gidx_v,
            "gvinv": 1.0 - gv,
            "sel": sel.astype(bfd),
            "kval01": kval.astype(np.float32).reshape(ET, 128).T.copy(),
            "fk01": mask[toks].astype(np.float32)
                      .reshape(QT, 128).T.copy(),
            "glb1m": (~is_glb[toks]).astype(np.float32)
                       .reshape(QT, 128).T.copy(),
        }
        m.update(shared)
        maps.append(m)
    return maps, M, cnt


def prepare(inputs):
    maps, M, cnt = _prep_maps(inputs)
    nc = _get_program(M, cnt)
    return nc, maps


def kernel(**inputs):
    nc, maps = prepare(inputs)
    res = run_bass_kernel_spmd(nc, maps, list(range(NC_)))
    out = np.concatenate([res.results[c]["out"] for c in range(NC_)], axis=0)
    return out[None].astype(np.float32)
